# revision 40
# baseline (speedup 1.0000x reference)
"""Trainium2 Bass kernel for nn_Attn: out = softmax_s(v . (W @ q_s + b)).

Algebraic identity:
    energies[s] = v . (W @ q[s] + b) = q[s] . (W^T v) + (v . b)
The (v . b) term is constant and softmax is shift-invariant, so it drops out.
The kernel computes u = W^T v (tiny), energies = question @ u (a matvec), and
a sharded softmax.

Engine choice: TensorE streams rhs at 1 column/SBUF-cycle (2-pass for fp32),
so pushing all of q through it costs ~75+ us/core. The DVE does a fused
multiply + free-axis reduce (scalar_tensor_tensor with accum_out) at
1 elem/lane/cycle (2/cycle for 16-bit), hiding under the DMA stream.

Precision: q and W stream as fp16 (host-cast), u is rounded to fp16; the
accumulations (PSUM for u, the stt accum_out for energies, softmax) stay
fp32. Error budget: W entries are ~N(0, 1/1024) so fp16 rounding of q/W/u
contributes ~8e-3 absolute energy error -> ~1% relative output error,
comfortably under the 2e-2 gate (and deterministic for the fixed test seed).
This halves the HBM stream (the hard floor) and halves PE pass count.

Why NO collectives: on this runner the 8 NEFFs enter ~60 us apart (the entry
barrier in every traced run spans to ~65 us), so ANY cross-core exchange
stalls early cores by the skew. Instead every core reads the full W and
computes u itself; cores are fully independent.

Queue discipline (learned the hard way): HWDGE DMA instructions occupy the
issuing engine's queue in program order, so every ACT compute op (u copies,
reduces, exp) must be EMITTED after the ACT-queue DMAs it will run among;
anything compute queued behind bulk DMAs it doesn't depend on stalls for
tens of us. The wv pieces lead both queues; 4 q chunks follow on each.

Distribution over 8 NeuronCores — seq (token) sharding, q in its NATIVE
[tokens, H] layout: core r owns tokens [r*4096, (r+1)*4096); partition p
holds tokens [32p, 32p+32), so every DMA is 128 partitions x contiguous
bytes. Per-PARTITION softmax stats (negmax via DVE reduce, exp + rowsum via
one ACT activation) are packed with the 32 unnormalized exp columns into one
[128, 34] output DMA; the host does the standard sharded-softmax merge
(M = max m_rp, Sg = sum s_rp*exp(m_rp-M), out = p_un * exp(m_rp-M)/Sg).
"""

import numpy as np

S = 32768
H = 1024
NCORES = 8
TPC = S // NCORES  # 4096 tokens per core
TPT = 32  # tokens (sub-tiles) per partition
NCHUNK = 8  # q DMAs per core (1 MB fp16 each)
SPC = TPT // NCHUNK  # sub-tiles per chunk (4)
OC = H // 128  # 8 o-chunks for the u matmuls

_cached = {}


def _build():
    from contextlib import ExitStack

    import concourse.bass as bass
    import concourse.mybir as mybir
    import concourse.tile as tile
    from concourse import bacc

    f32 = mybir.dt.float32
    f16 = mybir.dt.float16
    AX = mybir.AxisListType
    OP = mybir.AluOpType
    ds = bass.ds

    nc = bacc.Bacc(
        "TRN2", target_bir_lowering=False, debug=False, num_devices=NCORES
    )

    q = nc.dram_tensor("q", [TPC, H], f16, kind="ExternalInput")
    # wv = [vb | wcat]: vb (v replicated for the rank-1 lhsT) rides at the
    # head of the W stream — no separate tiny-descriptor v DMA
    wv = nc.dram_tensor("wv", [128, OC * 128 + OC * H], f16,
                        kind="ExternalInput")
    outp = nc.dram_tensor("outp", [128, TPT + 2], f32, kind="ExternalOutput")

    with tile.TileContext(nc) as tc, ExitStack() as ctx:
        const = ctx.enter_context(tc.tile_pool(name="const", bufs=1))
        qpool = ctx.enter_context(tc.tile_pool(name="qpool", bufs=NCHUNK))
        work = ctx.enter_context(tc.tile_pool(name="work", bufs=1))
        scr = ctx.enter_context(tc.tile_pool(name="scr", bufs=2))
        psum_b = ctx.enter_context(tc.tile_pool(name="psum_b", bufs=2, space="PSUM"))

        # --- wv pieces lead BOTH queues (vb + W-chunks 0,1 in piece 0);
        # q chunks follow, 4 per queue
        VBW = OC * 128  # 1024 cols of vb
        wv_sb = const.tile([128, VBW + OC * H], f16)
        wv_bounds = [0, VBW + H, VBW + 3 * H, VBW + 5 * H, VBW + OC * H]
        for w in range(4):
            eng = nc.scalar if w < 2 else nc.sync
            lo, hi = wv_bounds[w], wv_bounds[w + 1]
            eng.dma_start(wv_sb[:, ds(lo, hi - lo)], wv[:, ds(lo, hi - lo)])

        q_view = q[:].rearrange("(p t) h -> p (t h)", p=128)
        CW = SPC * H
        q_sb = []
        for k in range(NCHUNK):
            t_ = qpool.tile([128, CW], f16, tag="q")
            eng = nc.sync if k % 2 == 0 else nc.scalar
            eng.dma_start(t_[:], q_view[:, ds(k * CW, CW)])
            q_sb.append(t_)

        # --- u_rep = (v-replicated)^T @ W on the PE (fp16 1-pass matmuls):
        # the rank-1 vb lhsT writes u broadcast to ALL 128 output partitions,
        # so two PSUM banks accumulate the two 512-wide halves of u_rep
        # directly — no separate broadcast stage.
        pb0 = psum_b.tile([128, 512], f32, tag="pb0")
        pb1 = psum_b.tile([128, 512], f32, tag="pb1")
        pb = [pb0, pb1]
        for c in range(OC):
            for half in range(2):
                nc.tensor.matmul(
                    pb[half][:],
                    lhsT=wv_sb[:, ds(c * 128, 128)],
                    rhs=wv_sb[:, ds(VBW + c * H + half * 512, 512)],
                    start=(c == 0),
                    stop=(c == OC - 1),
                )
        # round u to fp16 on the (free) ACT engine — its DMA queue drained
        # ~20 us before this runs
        u_rep = const.tile([128, H], f16)
        for half in range(2):
            nc.scalar.copy(u_rep[:, ds(half * 512, 512)], pb[half][:])

        # --- energies, split across DVE and ACT:
        #   odd tiles (16): fused scalar_tensor_tensor on DVE (~1.22 us — the
        #     fused op has no fp16 2x mode but needs no second instruction);
        #     odd-fused puts tile 31 on DVE, so ACT's trailing reduce (tile
        #     30) finishes under DVE's shadow and never gates the softmax
        #   even tiles (16): tensor_tensor mult on DVE (2x_1p, ~0.6 us) + the
        #     reduce on ACT (activation Copy + accum_out, ~1.15 us)
        # -> DVE ~26 us, ACT ~23 us, overlapped.
        e_loc = work.tile([128, TPT], f32)
        for k in range(NCHUNK):
            for s_ in range(SPC):
                t_idx = k * SPC + s_
                if t_idx % 2 == 1:
                    prod = scr.tile([128, H], f16, tag="prod", bufs=2)
                    nc.vector.scalar_tensor_tensor(
                        out=prod[:], in0=q_sb[k][:, ds(s_ * H, H)], scalar=1.0,
                        in1=u_rep[:], op0=OP.mult, op1=OP.mult,
                        accum_out=e_loc[:, ds(t_idx, 1)],
                    )
                else:
                    prod = scr.tile([128, H], f16, tag="proda", bufs=4)
                    nc.vector.tensor_tensor(
                        prod[:], q_sb[k][:, ds(s_ * H, H)], u_rep[:],
                        op=OP.mult,
                    )
                    junk = scr.tile([128, H], f16, tag="junk", bufs=2)
                    nc.scalar.activation(
                        junk[:], prod[:], mybir.ActivationFunctionType.Copy,
                        accum_out=e_loc[:, ds(t_idx, 1)],
                    )

        # --- per-partition softmax pieces, packed with stats
        ot = work.tile([128, TPT + 2], f32)
        nc.vector.tensor_reduce(
            ot[:, ds(TPT, 1)], e_loc[:], axis=AX.X, op=OP.max, negate=True
        )
        nc.scalar.activation(
            ot[:, ds(0, TPT)], e_loc[:], mybir.ActivationFunctionType.Exp,
            bias=ot[:, ds(TPT, 1)], scale=1.0, accum_out=ot[:, ds(TPT + 1, 1)],
        )
        nc.sync.dma_start(outp[:], ot[:])

    nc.compile()
    return nc


def _get_nc():
    if "nc" not in _cached:
        _cached["nc"] = _build()
    return _cached["nc"]


def make_in_maps(question, W, v):
    qn = np.asarray(question)
    Wn = np.ascontiguousarray(np.asarray(W, dtype=np.float32))
    vn = np.ascontiguousarray(np.asarray(v, dtype=np.float32))
    q16 = np.ascontiguousarray(qn.astype(np.float16))
    # wcat[o, oc*H + j] = W[oc*128 + o, j] -> DMA is 128 x 16 KB contiguous
    wcat = (
        Wn.reshape(OC, 128, H).transpose(1, 0, 2).reshape(128, OC * H)
        .astype(np.float16)
    )
    # vb[o, c*128 + p] = v[128c + o], replicated across the 128 p-columns
    vtf = vn.reshape(OC, 128).T.astype(np.float16)  # [o, c]
    vb = np.broadcast_to(vtf[:, :, None], (128, OC, 128)).reshape(128, OC * 128)
    wvm = np.ascontiguousarray(np.concatenate([vb, wcat], axis=1))
    in_maps = []
    for r in range(NCORES):
        in_maps.append(
            {
                "q": q16[r * TPC : (r + 1) * TPC],  # contiguous row-slice view
                "wv": wvm,
            }
        )
    return in_maps


def run(question, W, v, **spmd_kwargs):
    """Run the SPMD kernel; returns (out [S] fp32, BassKernelResults)."""
    from concourse.bass_utils import run_bass_kernel_spmd

    nc = _get_nc()
    in_maps = make_in_maps(question, W, v)
    res = run_bass_kernel_spmd(nc, in_maps, core_ids=list(range(NCORES)), **spmd_kwargs)
    blocks = np.stack(
        [
            np.asarray(res.results[r]["outp"], dtype=np.float64).reshape(
                128, TPT + 2
            )
            for r in range(NCORES)
        ]
    )  # [8, 128, 34]; token of (r, p, t) = r*4096 + 32p + t
    p_un = blocks[:, :, :TPT]
    m = -blocks[:, :, TPT]
    sums = blocks[:, :, TPT + 1]
    M = m.max()
    wgt = np.exp(m - M)
    Sg = (sums * wgt).sum()
    out = (p_un * (wgt / Sg)[:, :, None]).reshape(S)
    return out.astype(np.float32), res


def kernel(question, W, b, v):
    out, _ = run(question, W, v)
    return out.reshape(1, 1, S)


# revision 41
# speedup vs baseline: 1.1330x; 1.1330x over previous
"""Trainium2 Bass kernel for nn_Attn: out = softmax_s(v . (W @ q_s + b)).

Algebraic identity:
    energies[s] = v . (W @ q[s] + b) = q[s] . (W^T v) + (v . b)
The (v . b) term is constant and softmax is shift-invariant, so it drops out.
The kernel computes u = W^T v (tiny), energies = question @ u (a matvec), and
a sharded softmax.

Engine choice: TensorE streams rhs at 1 column/SBUF-cycle (2-pass for fp32),
so pushing all of q through it costs ~75+ us/core. The DVE does a fused
multiply + free-axis reduce (scalar_tensor_tensor with accum_out) at
1 elem/lane/cycle (2/cycle for 16-bit), hiding under the DMA stream.

Precision: q and W stream as fp16 (host-cast), u is rounded to fp16; the
accumulations (PSUM for u, the stt accum_out for energies, softmax) stay
fp32. Error budget: W entries are ~N(0, 1/1024) so fp16 rounding of q/W/u
contributes ~8e-3 absolute energy error -> ~1% relative output error,
comfortably under the 2e-2 gate (and deterministic for the fixed test seed).
This halves the HBM stream (the hard floor) and halves PE pass count.

Why NO collectives: on this runner the 8 NEFFs enter ~60 us apart (the entry
barrier in every traced run spans to ~65 us), so ANY cross-core exchange
stalls early cores by the skew. Instead every core reads the full W and
computes u itself; cores are fully independent.

Queue discipline (learned the hard way): HWDGE DMA instructions occupy the
issuing engine's queue in program order, so every ACT compute op (u copies,
reduces, exp) must be EMITTED after the ACT-queue DMAs it will run among;
anything compute queued behind bulk DMAs it doesn't depend on stalls for
tens of us. The wv pieces lead both queues; 4 q chunks follow on each.

Distribution over 8 NeuronCores — seq (token) sharding, q in its NATIVE
[tokens, H] layout: core r owns tokens [r*4096, (r+1)*4096); partition p
holds tokens [32p, 32p+32), so every DMA is 128 partitions x contiguous
bytes. Per-PARTITION softmax stats (negmax via DVE reduce, exp + rowsum via
one ACT activation) are packed with the 32 unnormalized exp columns into one
[128, 34] output DMA; the host does the standard sharded-softmax merge
(M = max m_rp, Sg = sum s_rp*exp(m_rp-M), out = p_un * exp(m_rp-M)/Sg).
"""

import numpy as np

S = 32768
H = 1024
NCORES = 8
TPC = S // NCORES  # 4096 tokens per core
TPT = 32  # tokens (sub-tiles) per partition
NCHUNK = 8  # q DMAs per core (1 MB fp16 each)
SPC = TPT // NCHUNK  # sub-tiles per chunk (4)
OC = H // 128  # 8 o-chunks for the u matmuls

_cached = {}


def _build():
    from contextlib import ExitStack

    import concourse.bass as bass
    import concourse.mybir as mybir
    import concourse.tile as tile
    from concourse import bacc

    f32 = mybir.dt.float32
    f16 = mybir.dt.float16
    AX = mybir.AxisListType
    OP = mybir.AluOpType
    ds = bass.ds

    nc = bacc.Bacc(
        "TRN2", target_bir_lowering=False, debug=False, num_devices=NCORES
    )

    q = nc.dram_tensor("q", [TPC, H], f16, kind="ExternalInput")
    # wv = [vb | wcat]: vb (v replicated for the rank-1 lhsT) rides at the
    # head of the W stream — no separate tiny-descriptor v DMA
    wv = nc.dram_tensor("wv", [128, OC * 128 + OC * H], f16,
                        kind="ExternalInput")
    outp = nc.dram_tensor("outp", [128, TPT + 2], f32, kind="ExternalOutput")

    with tile.TileContext(nc) as tc, ExitStack() as ctx:
        const = ctx.enter_context(tc.tile_pool(name="const", bufs=1))
        qpool = ctx.enter_context(tc.tile_pool(name="qpool", bufs=NCHUNK))
        work = ctx.enter_context(tc.tile_pool(name="work", bufs=1))
        scr = ctx.enter_context(tc.tile_pool(name="scr", bufs=2))
        psum_b = ctx.enter_context(tc.tile_pool(name="psum_b", bufs=2, space="PSUM"))

        # --- wv pieces lead BOTH queues (vb + W-chunks 0,1 in piece 0);
        # q chunks follow, 4 per queue
        VBW = OC * 128  # 1024 cols of vb
        wv_sb = const.tile([128, VBW + OC * H], f16)
        wv_bounds = [0, VBW + H, VBW + 3 * H, VBW + 5 * H, VBW + OC * H]
        for w in range(4):
            eng = nc.scalar if w < 2 else nc.sync
            lo, hi = wv_bounds[w], wv_bounds[w + 1]
            eng.dma_start(wv_sb[:, ds(lo, hi - lo)], wv[:, ds(lo, hi - lo)])

        q_view = q[:].rearrange("(p t) h -> p (t h)", p=128)
        CW = SPC * H
        q_sb = []
        for k in range(NCHUNK):
            t_ = qpool.tile([128, CW], f16, tag="q")
            eng = nc.sync if k % 2 == 0 else nc.scalar
            eng.dma_start(t_[:], q_view[:, ds(k * CW, CW)])
            q_sb.append(t_)

        # --- u_rep = (v-replicated)^T @ W on the PE (fp16 1-pass matmuls):
        # the rank-1 vb lhsT writes u broadcast to ALL 128 output partitions,
        # so two PSUM banks accumulate the two 512-wide halves of u_rep
        # directly — no separate broadcast stage.
        pb0 = psum_b.tile([128, 512], f32, tag="pb0")
        pb1 = psum_b.tile([128, 512], f32, tag="pb1")
        pb = [pb0, pb1]
        for c in range(OC):
            for half in range(2):
                nc.tensor.matmul(
                    pb[half][:],
                    lhsT=wv_sb[:, ds(c * 128, 128)],
                    rhs=wv_sb[:, ds(VBW + c * H + half * 512, 512)],
                    start=(c == 0),
                    stop=(c == OC - 1),
                )
        # round u to fp16 on the (free) ACT engine — its DMA queue drained
        # ~20 us before this runs
        u_rep = const.tile([128, H], f16)
        for half in range(2):
            nc.scalar.copy(u_rep[:, ds(half * 512, 512)], pb[half][:])

        # --- energies, split across DVE and ACT:
        #   even tiles (16): fused scalar_tensor_tensor on DVE (~1.22 us — the
        #     fused op has no fp16 2x mode but needs no second instruction)
        #   odd tiles (16): tensor_tensor mult on DVE (2x_1p, ~0.6 us) + the
        #     reduce on ACT (activation Copy + accum_out, ~1.4 us)
        # -> DVE ~26 us, ACT ~23 us, overlapped.
        e_loc = work.tile([128, TPT], f32)
        for k in range(NCHUNK):
            for s_ in range(SPC):
                t_idx = k * SPC + s_
                if t_idx % 2 == 0:
                    prod = scr.tile([128, H], f16, tag="prod", bufs=2)
                    nc.vector.scalar_tensor_tensor(
                        out=prod[:], in0=q_sb[k][:, ds(s_ * H, H)], scalar=1.0,
                        in1=u_rep[:], op0=OP.mult, op1=OP.mult,
                        accum_out=e_loc[:, ds(t_idx, 1)],
                    )
                else:
                    prod = scr.tile([128, H], f16, tag="proda", bufs=4)
                    nc.vector.tensor_tensor(
                        prod[:], q_sb[k][:, ds(s_ * H, H)], u_rep[:],
                        op=OP.mult,
                    )
                    junk = scr.tile([128, H], f16, tag="junk", bufs=2)
                    nc.scalar.activation(
                        junk[:], prod[:], mybir.ActivationFunctionType.Copy,
                        accum_out=e_loc[:, ds(t_idx, 1)],
                    )

        # --- per-partition softmax pieces, packed with stats
        ot = work.tile([128, TPT + 2], f32)
        nc.vector.tensor_reduce(
            ot[:, ds(TPT, 1)], e_loc[:], axis=AX.X, op=OP.max, negate=True
        )
        nc.scalar.activation(
            ot[:, ds(0, TPT)], e_loc[:], mybir.ActivationFunctionType.Exp,
            bias=ot[:, ds(TPT, 1)], scale=1.0, accum_out=ot[:, ds(TPT + 1, 1)],
        )
        nc.sync.dma_start(outp[:], ot[:])

    nc.compile()
    return nc


def _get_nc():
    if "nc" not in _cached:
        _cached["nc"] = _build()
    return _cached["nc"]


def make_in_maps(question, W, v):
    qn = np.asarray(question)
    Wn = np.ascontiguousarray(np.asarray(W, dtype=np.float32))
    vn = np.ascontiguousarray(np.asarray(v, dtype=np.float32))
    q16 = np.ascontiguousarray(qn.astype(np.float16))
    # wcat[o, oc*H + j] = W[oc*128 + o, j] -> DMA is 128 x 16 KB contiguous
    wcat = (
        Wn.reshape(OC, 128, H).transpose(1, 0, 2).reshape(128, OC * H)
        .astype(np.float16)
    )
    # vb[o, c*128 + p] = v[128c + o], replicated across the 128 p-columns
    vtf = vn.reshape(OC, 128).T.astype(np.float16)  # [o, c]
    vb = np.broadcast_to(vtf[:, :, None], (128, OC, 128)).reshape(128, OC * 128)
    wvm = np.ascontiguousarray(np.concatenate([vb, wcat], axis=1))
    in_maps = []
    for r in range(NCORES):
        in_maps.append(
            {
                "q": q16[r * TPC : (r + 1) * TPC],  # contiguous row-slice view
                "wv": wvm,
            }
        )
    return in_maps


def run(question, W, v, **spmd_kwargs):
    """Run the SPMD kernel; returns (out [S] fp32, BassKernelResults)."""
    from concourse.bass_utils import run_bass_kernel_spmd

    nc = _get_nc()
    in_maps = make_in_maps(question, W, v)
    res = run_bass_kernel_spmd(nc, in_maps, core_ids=list(range(NCORES)), **spmd_kwargs)
    blocks = np.stack(
        [
            np.asarray(res.results[r]["outp"], dtype=np.float64).reshape(
                128, TPT + 2
            )
            for r in range(NCORES)
        ]
    )  # [8, 128, 34]; token of (r, p, t) = r*4096 + 32p + t
    p_un = blocks[:, :, :TPT]
    m = -blocks[:, :, TPT]
    sums = blocks[:, :, TPT + 1]
    M = m.max()
    wgt = np.exp(m - M)
    Sg = (sums * wgt).sum()
    out = (p_un * (wgt / Sg)[:, :, None]).reshape(S)
    return out.astype(np.float32), res


def kernel(question, W, b, v):
    out, _ = run(question, W, v)
    return out.reshape(1, 1, S)


# revision 43
# speedup vs baseline: 1.1657x; 1.0289x over previous
"""Trainium2 Bass kernel for nn_Attn: out = softmax_s(v . (W @ q_s + b)).

Algebraic identity:
    energies[s] = v . (W @ q[s] + b) = q[s] . (W^T v) + (v . b)
The (v . b) term is constant and softmax is shift-invariant, so it drops out.
The kernel computes u = W^T v (tiny), energies = question @ u (a matvec), and
a sharded softmax.

Engine choice: TensorE streams rhs at 1 column/SBUF-cycle (2-pass for fp32),
so pushing all of q through it costs ~75+ us/core. The DVE does a fused
multiply + free-axis reduce (scalar_tensor_tensor with accum_out) at
1 elem/lane/cycle (2/cycle for 16-bit), hiding under the DMA stream.

Precision: q and W stream as fp16 (host-cast), u is rounded to fp16; the
accumulations (PSUM for u, the stt accum_out for energies, softmax) stay
fp32. Error budget: W entries are ~N(0, 1/1024) so fp16 rounding of q/W/u
contributes ~8e-3 absolute energy error -> ~1% relative output error,
comfortably under the 2e-2 gate (and deterministic for the fixed test seed).
This halves the HBM stream (the hard floor) and halves PE pass count.

Why NO collectives: on this runner the 8 NEFFs enter ~60 us apart (the entry
barrier in every traced run spans to ~65 us), so ANY cross-core exchange
stalls early cores by the skew. Instead every core reads the full W and
computes u itself; cores are fully independent.

Queue discipline (learned the hard way): HWDGE DMA instructions occupy the
issuing engine's queue in program order, so every ACT compute op (u copies,
reduces, exp) must be EMITTED after the ACT-queue DMAs it will run among;
anything compute queued behind bulk DMAs it doesn't depend on stalls for
tens of us. The wv pieces lead both queues; 4 q chunks follow on each.

Distribution over 8 NeuronCores — seq (token) sharding, q in its NATIVE
[tokens, H] layout: core r owns tokens [r*4096, (r+1)*4096); partition p
holds tokens [32p, 32p+32), so every DMA is 128 partitions x contiguous
bytes. Per-PARTITION softmax stats (negmax via DVE reduce, exp + rowsum via
one ACT activation) are packed with the 32 unnormalized exp columns into one
[128, 34] output DMA; the host does the standard sharded-softmax merge
(M = max m_rp, Sg = sum s_rp*exp(m_rp-M), out = p_un * exp(m_rp-M)/Sg).
"""

import numpy as np

S = 32768
H = 1024
NCORES = 8
TPC = S // NCORES  # 4096 tokens per core
TPT = 32  # tokens (sub-tiles) per partition
NCHUNK = 8  # q DMAs per core (1 MB fp16 each)
SPC = TPT // NCHUNK  # sub-tiles per chunk (4)
OC = H // 128  # 8 o-chunks for the u matmuls

_cached = {}


def _build():
    from contextlib import ExitStack

    import concourse.bass as bass
    import concourse.mybir as mybir
    import concourse.tile as tile
    from concourse import bacc

    f32 = mybir.dt.float32
    f16 = mybir.dt.float16
    AX = mybir.AxisListType
    OP = mybir.AluOpType
    ds = bass.ds

    nc = bacc.Bacc(
        "TRN2", target_bir_lowering=False, debug=False, num_devices=NCORES
    )

    q = nc.dram_tensor("q", [TPC, H], f16, kind="ExternalInput")
    # wv = [vb | wcat]: vb (v replicated for the rank-1 lhsT) rides at the
    # head of the W stream — no separate tiny-descriptor v DMA
    wv = nc.dram_tensor("wv", [128, OC * 128 + OC * H], f16,
                        kind="ExternalInput")
    outp = nc.dram_tensor("outp", [128, TPT + 2], f32, kind="ExternalOutput")

    with tile.TileContext(nc) as tc, ExitStack() as ctx:
        const = ctx.enter_context(tc.tile_pool(name="const", bufs=1))
        qpool = ctx.enter_context(tc.tile_pool(name="qpool", bufs=NCHUNK))
        work = ctx.enter_context(tc.tile_pool(name="work", bufs=1))
        scr = ctx.enter_context(tc.tile_pool(name="scr", bufs=2))
        psum_b = ctx.enter_context(tc.tile_pool(name="psum_b", bufs=2, space="PSUM"))

        # --- wv pieces lead BOTH queues (vb + W-chunks 0,1 in piece 0);
        # q chunks follow, 4 per queue
        VBW = OC * 128  # 1024 cols of vb
        wv_sb = const.tile([128, VBW + OC * H], f16)
        wv_bounds = [0, VBW + H, VBW + 3 * H, VBW + 5 * H, VBW + OC * H]
        for w in range(4):
            eng = nc.scalar if w < 2 else nc.sync
            lo, hi = wv_bounds[w], wv_bounds[w + 1]
            eng.dma_start(wv_sb[:, ds(lo, hi - lo)], wv[:, ds(lo, hi - lo)])

        q_view = q[:].rearrange("(p t) h -> p (t h)", p=128)
        CW = SPC * H
        q_sb = []
        for k in range(NCHUNK):
            t_ = qpool.tile([128, CW], f16, tag="q")
            eng = nc.sync if k % 2 == 0 else nc.scalar
            eng.dma_start(t_[:], q_view[:, ds(k * CW, CW)])
            q_sb.append(t_)

        # --- u_rep = (v-replicated)^T @ W on the PE (fp16 1-pass matmuls):
        # the rank-1 vb lhsT writes u broadcast to ALL 128 output partitions,
        # so two PSUM banks accumulate the two 512-wide halves of u_rep
        # directly — no separate broadcast stage.
        pb0 = psum_b.tile([128, 512], f32, tag="pb0")
        pb1 = psum_b.tile([128, 512], f32, tag="pb1")
        pb = [pb0, pb1]
        for c in range(OC):
            for half in range(2):
                nc.tensor.matmul(
                    pb[half][:],
                    lhsT=wv_sb[:, ds(c * 128, 128)],
                    rhs=wv_sb[:, ds(VBW + c * H + half * 512, 512)],
                    start=(c == 0),
                    stop=(c == OC - 1),
                )
        # round u to fp16 on the (free) ACT engine — its DMA queue drained
        # ~20 us before this runs
        u_rep = const.tile([128, H], f16)
        for half in range(2):
            nc.scalar.copy(u_rep[:, ds(half * 512, 512)], pb[half][:])

        # --- energies, split across DVE and ACT:
        #   even tiles (16): fused scalar_tensor_tensor on DVE (~1.22 us — the
        #     fused op has no fp16 2x mode but needs no second instruction)
        #   odd tiles (16): tensor_tensor mult on DVE (2x_1p, ~0.6 us) + the
        #     reduce on ACT (activation Copy + accum_out, ~1.4 us)
        # -> DVE ~26 us, ACT ~23 us, overlapped.
        e_loc = work.tile([128, TPT], f32)
        for k in range(NCHUNK):
            for s_ in range(SPC):
                t_idx = k * SPC + s_
                if t_idx % 2 == 0:
                    prod = scr.tile([128, H], f16, tag="prod", bufs=2)
                    nc.vector.scalar_tensor_tensor(
                        out=prod[:], in0=q_sb[k][:, ds(s_ * H, H)], scalar=1.0,
                        in1=u_rep[:], op0=OP.mult, op1=OP.mult,
                        accum_out=e_loc[:, ds(t_idx, 1)],
                    )
                else:
                    prod = scr.tile([128, H], f16, tag="proda", bufs=4)
                    nc.vector.tensor_tensor(
                        prod[:], q_sb[k][:, ds(s_ * H, H)], u_rep[:],
                        op=OP.mult,
                    )
                    junk = scr.tile([128, H], f16, tag="junk", bufs=2)
                    nc.scalar.activation(
                        junk[:], prod[:], mybir.ActivationFunctionType.Copy,
                        accum_out=e_loc[:, ds(t_idx, 1)],
                    )

        # --- per-partition softmax pieces, packed with stats
        ot = work.tile([128, TPT + 2], f32)
        nc.vector.tensor_reduce(
            ot[:, ds(TPT, 1)], e_loc[:], axis=AX.X, op=OP.max, negate=True
        )
        nc.scalar.activation(
            ot[:, ds(0, TPT)], e_loc[:], mybir.ActivationFunctionType.Exp,
            bias=ot[:, ds(TPT, 1)], scale=1.0, accum_out=ot[:, ds(TPT + 1, 1)],
        )
        nc.sync.dma_start(outp[:], ot[:])

    nc.compile()
    return nc


def _get_nc():
    if "nc" not in _cached:
        _cached["nc"] = _build()
    return _cached["nc"]


def make_in_maps(question, W, v):
    qn = np.asarray(question)
    Wn = np.ascontiguousarray(np.asarray(W, dtype=np.float32))
    vn = np.ascontiguousarray(np.asarray(v, dtype=np.float32))
    q16 = np.ascontiguousarray(qn.astype(np.float16))
    # wcat[o, oc*H + j] = W[oc*128 + o, j] -> DMA is 128 x 16 KB contiguous
    wcat = (
        Wn.reshape(OC, 128, H).transpose(1, 0, 2).reshape(128, OC * H)
        .astype(np.float16)
    )
    # vb[o, c*128 + p] = v[128c + o], replicated across the 128 p-columns
    vtf = vn.reshape(OC, 128).T.astype(np.float16)  # [o, c]
    vb = np.broadcast_to(vtf[:, :, None], (128, OC, 128)).reshape(128, OC * 128)
    wvm = np.ascontiguousarray(np.concatenate([vb, wcat], axis=1))
    in_maps = []
    for r in range(NCORES):
        in_maps.append(
            {
                "q": q16[r * TPC : (r + 1) * TPC],  # contiguous row-slice view
                "wv": wvm,
            }
        )
    return in_maps


def run(question, W, v, **spmd_kwargs):
    """Run the SPMD kernel; returns (out [S] fp32, BassKernelResults)."""
    from concourse.bass_utils import run_bass_kernel_spmd

    nc = _get_nc()
    in_maps = make_in_maps(question, W, v)
    res = run_bass_kernel_spmd(nc, in_maps, core_ids=list(range(NCORES)), **spmd_kwargs)
    blocks = np.stack(
        [
            np.asarray(res.results[r]["outp"], dtype=np.float64).reshape(
                128, TPT + 2
            )
            for r in range(NCORES)
        ]
    )  # [8, 128, 34]; token of (r, p, t) = r*4096 + 32p + t
    p_un = blocks[:, :, :TPT]
    m = -blocks[:, :, TPT]
    sums = blocks[:, :, TPT + 1]
    M = m.max()
    wgt = np.exp(m - M)
    Sg = (sums * wgt).sum()
    out = (p_un * (wgt / Sg)[:, :, None]).reshape(S)
    return out.astype(np.float32), res


def kernel(question, W, b, v):
    out, _ = run(question, W, v)
    return out.reshape(1, 1, S)


# revision 44
# speedup vs baseline: 1.1688x; 1.0027x over previous
"""Trainium2 Bass kernel for nn_Attn: out = softmax_s(v . (W @ q_s + b)).

Algebraic identity:
    energies[s] = v . (W @ q[s] + b) = q[s] . (W^T v) + (v . b)
The (v . b) term is constant and softmax is shift-invariant, so it drops out.
The kernel computes u = W^T v (tiny), energies = question @ u (a matvec), and
a sharded softmax.

Engine choice: TensorE streams rhs at 1 column/SBUF-cycle (2-pass for fp32),
so pushing all of q through it costs ~75+ us/core. The DVE does a fused
multiply + free-axis reduce (scalar_tensor_tensor with accum_out) at
1 elem/lane/cycle (2/cycle for 16-bit), hiding under the DMA stream.

Precision: q and W stream as fp16 (host-cast), u is rounded to fp16; the
accumulations (PSUM for u, the stt accum_out for energies, softmax) stay
fp32. Error budget: W entries are ~N(0, 1/1024) so fp16 rounding of q/W/u
contributes ~8e-3 absolute energy error -> ~1% relative output error,
comfortably under the 2e-2 gate (and deterministic for the fixed test seed).
This halves the HBM stream (the hard floor) and halves PE pass count.

Why NO collectives: on this runner the 8 NEFFs enter ~60 us apart (the entry
barrier in every traced run spans to ~65 us), so ANY cross-core exchange
stalls early cores by the skew. Instead every core reads the full W and
computes u itself; cores are fully independent.

Queue discipline (learned the hard way): HWDGE DMA instructions occupy the
issuing engine's queue in program order, so every ACT compute op (u copies,
reduces, exp) must be EMITTED after the ACT-queue DMAs it will run among;
anything compute queued behind bulk DMAs it doesn't depend on stalls for
tens of us. The wv pieces lead both queues; 4 q chunks follow on each.

Distribution over 8 NeuronCores — seq (token) sharding, q in its NATIVE
[tokens, H] layout: core r owns tokens [r*4096, (r+1)*4096); partition p
holds tokens [32p, 32p+32), so every DMA is 128 partitions x contiguous
bytes. Per-PARTITION softmax stats (negmax via DVE reduce, exp + rowsum via
one ACT activation) are packed with the 32 unnormalized exp columns into one
[128, 34] output DMA; the host does the standard sharded-softmax merge
(M = max m_rp, Sg = sum s_rp*exp(m_rp-M), out = p_un * exp(m_rp-M)/Sg).
"""

import numpy as np

S = 32768
H = 1024
NCORES = 8
TPC = S // NCORES  # 4096 tokens per core
TPT = 32  # tokens (sub-tiles) per partition
NCHUNK = 8  # q DMAs per core (1 MB fp16 each)
SPC = TPT // NCHUNK  # sub-tiles per chunk (4)
OC = H // 128  # 8 o-chunks for the u matmuls

_cached = {}


def _build():
    from contextlib import ExitStack

    import concourse.bass as bass
    import concourse.mybir as mybir
    import concourse.tile as tile
    from concourse import bacc

    f32 = mybir.dt.float32
    f16 = mybir.dt.float16
    AX = mybir.AxisListType
    OP = mybir.AluOpType
    ds = bass.ds

    nc = bacc.Bacc(
        "TRN2", target_bir_lowering=False, debug=False, num_devices=NCORES
    )

    q = nc.dram_tensor("q", [TPC, H], f16, kind="ExternalInput")
    # wv = [vb | wcat]: vb (v replicated for the rank-1 lhsT) rides at the
    # head of the W stream — no separate tiny-descriptor v DMA
    wv = nc.dram_tensor("wv", [128, OC * 128 + OC * H], f16,
                        kind="ExternalInput")
    outp = nc.dram_tensor("outp", [128, TPT + 2], f32, kind="ExternalOutput")

    with tile.TileContext(nc) as tc, ExitStack() as ctx:
        const = ctx.enter_context(tc.tile_pool(name="const", bufs=1))
        qpool = ctx.enter_context(tc.tile_pool(name="qpool", bufs=NCHUNK))
        work = ctx.enter_context(tc.tile_pool(name="work", bufs=1))
        scr = ctx.enter_context(tc.tile_pool(name="scr", bufs=2))
        psum_b = ctx.enter_context(tc.tile_pool(name="psum_b", bufs=2, space="PSUM"))

        # --- wv pieces lead BOTH queues (vb + W-chunks 0,1 in piece 0);
        # q chunks follow, 4 per queue
        VBW = OC * 128  # 1024 cols of vb
        wv_sb = const.tile([128, VBW + OC * H], f16)
        wv_bounds = [0, VBW + H, VBW + 3 * H, VBW + 5 * H, VBW + OC * H]
        for w in range(4):
            eng = nc.scalar if w < 2 else nc.sync
            lo, hi = wv_bounds[w], wv_bounds[w + 1]
            eng.dma_start(wv_sb[:, ds(lo, hi - lo)], wv[:, ds(lo, hi - lo)])

        q_view = q[:].rearrange("(p t) h -> p (t h)", p=128)
        CW = SPC * H
        q_sb = []
        for k in range(NCHUNK):
            t_ = qpool.tile([128, CW], f16, tag="q")
            eng = nc.sync if k % 2 == 0 else nc.scalar
            eng.dma_start(t_[:], q_view[:, ds(k * CW, CW)])
            q_sb.append(t_)

        # --- u_rep = (v-replicated)^T @ W on the PE (fp16 1-pass matmuls):
        # the rank-1 vb lhsT writes u broadcast to ALL 128 output partitions,
        # so two PSUM banks accumulate the two 512-wide halves of u_rep
        # directly — no separate broadcast stage.
        pb0 = psum_b.tile([128, 512], f32, tag="pb0")
        pb1 = psum_b.tile([128, 512], f32, tag="pb1")
        pb = [pb0, pb1]
        for c in range(OC):
            for half in range(2):
                nc.tensor.matmul(
                    pb[half][:],
                    lhsT=wv_sb[:, ds(c * 128, 128)],
                    rhs=wv_sb[:, ds(VBW + c * H + half * 512, 512)],
                    start=(c == 0),
                    stop=(c == OC - 1),
                )
        # round u to fp16 on the (free) ACT engine — its DMA queue drained
        # ~20 us before this runs
        u_rep = const.tile([128, H], f16)
        for half in range(2):
            nc.scalar.copy(u_rep[:, ds(half * 512, 512)], pb[half][:])

        # --- energies, split across DVE and ACT:
        #   odd tiles (16): fused scalar_tensor_tensor on DVE (~1.22 us — the
        #     fused op has no fp16 2x mode but needs no second instruction);
        #     odd-fused puts tile 31 on DVE, so ACT's trailing reduce (tile
        #     30) finishes under DVE's shadow and never gates the softmax
        #   even tiles (16): tensor_tensor mult on DVE (2x_1p, ~0.6 us) + the
        #     reduce on ACT (activation Copy + accum_out, ~1.15 us)
        # -> DVE ~26 us, ACT ~23 us, overlapped.
        e_loc = work.tile([128, TPT], f32)
        for k in range(NCHUNK):
            for s_ in range(SPC):
                t_idx = k * SPC + s_
                if t_idx % 2 == 1:
                    prod = scr.tile([128, H], f16, tag="prod", bufs=2)
                    nc.vector.scalar_tensor_tensor(
                        out=prod[:], in0=q_sb[k][:, ds(s_ * H, H)], scalar=1.0,
                        in1=u_rep[:], op0=OP.mult, op1=OP.mult,
                        accum_out=e_loc[:, ds(t_idx, 1)],
                    )
                else:
                    prod = scr.tile([128, H], f16, tag="proda", bufs=4)
                    nc.vector.tensor_tensor(
                        prod[:], q_sb[k][:, ds(s_ * H, H)], u_rep[:],
                        op=OP.mult,
                    )
                    junk = scr.tile([128, H], f16, tag="junk", bufs=2)
                    nc.scalar.activation(
                        junk[:], prod[:], mybir.ActivationFunctionType.Copy,
                        accum_out=e_loc[:, ds(t_idx, 1)],
                    )

        # --- per-partition softmax pieces, packed with stats
        ot = work.tile([128, TPT + 2], f32)
        nc.vector.tensor_reduce(
            ot[:, ds(TPT, 1)], e_loc[:], axis=AX.X, op=OP.max, negate=True
        )
        nc.scalar.activation(
            ot[:, ds(0, TPT)], e_loc[:], mybir.ActivationFunctionType.Exp,
            bias=ot[:, ds(TPT, 1)], scale=1.0, accum_out=ot[:, ds(TPT + 1, 1)],
        )
        nc.sync.dma_start(outp[:], ot[:])

    nc.compile()
    return nc


def _get_nc():
    if "nc" not in _cached:
        _cached["nc"] = _build()
    return _cached["nc"]


def make_in_maps(question, W, v):
    qn = np.asarray(question)
    Wn = np.ascontiguousarray(np.asarray(W, dtype=np.float32))
    vn = np.ascontiguousarray(np.asarray(v, dtype=np.float32))
    q16 = np.ascontiguousarray(qn.astype(np.float16))
    # wcat[o, oc*H + j] = W[oc*128 + o, j] -> DMA is 128 x 16 KB contiguous
    wcat = (
        Wn.reshape(OC, 128, H).transpose(1, 0, 2).reshape(128, OC * H)
        .astype(np.float16)
    )
    # vb[o, c*128 + p] = v[128c + o], replicated across the 128 p-columns
    vtf = vn.reshape(OC, 128).T.astype(np.float16)  # [o, c]
    vb = np.broadcast_to(vtf[:, :, None], (128, OC, 128)).reshape(128, OC * 128)
    wvm = np.ascontiguousarray(np.concatenate([vb, wcat], axis=1))
    in_maps = []
    for r in range(NCORES):
        in_maps.append(
            {
                "q": q16[r * TPC : (r + 1) * TPC],  # contiguous row-slice view
                "wv": wvm,
            }
        )
    return in_maps


def run(question, W, v, **spmd_kwargs):
    """Run the SPMD kernel; returns (out [S] fp32, BassKernelResults)."""
    from concourse.bass_utils import run_bass_kernel_spmd

    nc = _get_nc()
    in_maps = make_in_maps(question, W, v)
    res = run_bass_kernel_spmd(nc, in_maps, core_ids=list(range(NCORES)), **spmd_kwargs)
    blocks = np.stack(
        [
            np.asarray(res.results[r]["outp"], dtype=np.float64).reshape(
                128, TPT + 2
            )
            for r in range(NCORES)
        ]
    )  # [8, 128, 34]; token of (r, p, t) = r*4096 + 32p + t
    p_un = blocks[:, :, :TPT]
    m = -blocks[:, :, TPT]
    sums = blocks[:, :, TPT + 1]
    M = m.max()
    wgt = np.exp(m - M)
    Sg = (sums * wgt).sum()
    out = (p_un * (wgt / Sg)[:, :, None]).reshape(S)
    return out.astype(np.float32), res


def kernel(question, W, b, v):
    out, _ = run(question, W, v)
    return out.reshape(1, 1, S)
